# revision 29
# baseline (speedup 1.0000x reference)
"""Trainium2 Bass kernel for nn_ClassificationHead: LayerNorm -> Linear(1024,256) -> GELU -> Linear(256,2).

Data-parallel over 8 NeuronCores: each core processes 8192 rows of the
65536-row batch; the tiny weights are replicated. The host supplies each
core's shard pre-transposed in bf16 (layout-only prep: [1024, 8192] K-major
with a ones column appended per 128-row subtile for the on-device rowsum).

Per-core pipeline, per 128-row tile:
  1. TensorE: 8 accumulating matmuls x@W1' into a half-bank PSUM slot
     (PSUM tiles are allocated in pairs [128, 2, 256] = exactly one 2KB
     bank, doubling the in-flight tile lookahead to ~12 so the PE never
     stalls on the stats chain), plus 8 Gram matmuls reusing the same
     stationary x-chunk against [x|1] (129 cols) -> Gram + rowsum.
  2. DVE extracts -mu (Gram ones-col) and sum(x^2) (Gram diagonal).
  3. Per 4-tile block: var = SS/D - mu^2 + eps on DVE; rhat = ACT Sqrt
     (bf16), g = DVE reciprocal(rhat). A [128,128] xbar-transpose DMA
     flips (-mu, rhat) into rows.
  4. TensorE adds the rank-2 correction (-mu ox s1 + rhat ox c1), so after
     the GELU's per-partition scale g the PSUM holds exactly LN(x)@W1'+b1.
  5. ACT evaluates exact GELU with scale g -> bf16 h tile.
  6. DVE computes h @ W2 via two reduce dot-products; adds b2 at block end.
  7. One DMA writes the [8192, 2] fp32 result back.

Host-side weight folding (tiny, O(1MB)): W1' = ln_w[:,None]*W1,
s1 = colsum(W1'), c1 = ln_b@W1 + b1.
"""
import sys

sys.path.insert(0, "/opt/trn_rl_repo")
sys.path.insert(0, "/root/.axon_site")

import numpy as np
import ml_dtypes

N_CORES = 8
BATCH = 65536
D = 1024
H = 256
OUT = 2
RPC = BATCH // N_CORES  # rows per core
NT = RPC // 128         # 128-row tiles per core
KC = D // 128           # contraction chunks
G = 4                   # tiles per block (512 rows)
NB = NT // G            # blocks per core
EPS = 1e-5
MAGIC = 0x5F3759DF

_cache = {}


def _bf16(a):
    return np.asarray(a, dtype=ml_dtypes.bfloat16)


def _build(rpc=RPC):
    import concourse.bacc as bacc
    import concourse.mybir as mybir
    from concourse import tile

    f32 = mybir.dt.float32
    i32 = mybir.dt.int32
    bf16 = mybir.dt.bfloat16
    AF = mybir.ActivationFunctionType
    ALU = mybir.AluOpType

    nc = bacc.Bacc(None, target_bir_lowering=False, debug=False)

    nt = rpc // 128
    # Half-block-major so each load is one contiguous run per partition
    # (1KB-run slicing measured only ~12.8 GB/s per SDMA engine) and the
    # first tiles of a block can start after half the block has landed.
    xt_in = nc.dram_tensor(
        "xt", [nt // G, 2, 128, KC, 2, 129], bf16, kind="ExternalInput"
    )
    w1_in = nc.dram_tensor("w1b", [128, KC, H], bf16, kind="ExternalInput")
    sc_in = nc.dram_tensor("screp", [2 * G, G, H], bf16, kind="ExternalInput")
    w2_in = nc.dram_tensor("w2rep", [128, OUT, H], bf16, kind="ExternalInput")
    b2_in = nc.dram_tensor("b2g", [128, G * OUT], f32, kind="ExternalInput")
    idf_in = nc.dram_tensor("identf", [128, 128], f32, kind="ExternalInput")
    # Stored [p, t, c]; host untangles to [rpc, c] (contiguous DMA runs).
    y_out = nc.dram_tensor("y", [128, nt, OUT], f32, kind="ExternalOutput")
    y_v = y_out

    with tile.TileContext(nc) as tc:
        with (
            tc.tile_pool(name="wpool", bufs=1) as wp,
            tc.tile_pool(name="xtp", bufs=10) as xtp,
            tc.tile_pool(name="statp", bufs=2) as statp,
            tc.tile_pool(name="scrp", bufs=2) as scrp,
            tc.tile_pool(name="hbp", bufs=3) as hbp,
            tc.tile_pool(name="outp", bufs=1) as outp,
            tc.tile_pool(name="pszp", bufs=6, space="PSUM") as pszp,
            tc.tile_pool(name="psgp", bufs=2, space="PSUM") as psgp,
        ):
            # Prefetch the first block's activations before the weight
            # loads so the PE ramp isn't serialized behind them.
            xtg_pre = xtp.tile([128, KC, G, 129], bf16, tag="xtg")
            for h in range(2):
                nc.sync.dma_start(
                    xtg_pre[:, :, 2 * h : 2 * h + 2, :], xt_in[0, h]
                )

            w1sb = wp.tile([128, KC, H], bf16)
            nc.sync.dma_start(w1sb[:], w1_in[:])
            scsb = wp.tile([2 * G, G, H], bf16)
            nc.sync.dma_start(scsb[:], sc_in[:])
            w2sb = wp.tile([128, OUT, H], bf16)
            nc.sync.dma_start(w2sb[:], w2_in[:])
            b2sb = wp.tile([128, G * OUT], f32)
            nc.sync.dma_start(b2sb[:], b2_in[:])
            idfsb = wp.tile([128, 128], f32)
            nc.sync.dma_start(idfsb[:], idf_in[:])

            outsb = outp.tile([128, nt, OUT], f32)

            # Software pipeline: each block's back half (rank2 correction,
            # GELU, W2 dots) is EMITTED one block late so the in-order PE
            # queue never has a stats-dependent matmul ahead of the next
            # block's dense mm1/gram work (head-of-line blocking).
            def back_half(u, pszds, BMT, Y, OB):
                for q in range(G):
                    zslot = pszds[q // 2][:, q % 2, :]
                    nc.tensor.matmul(
                        zslot, BMT[0 : 2 * G, :],
                        scsb[:, q, :], start=False, stop=(q % 2 == 1),
                        skip_group_check=True,
                    )
                    hb = hbp.tile([128, H], bf16, tag="hb")
                    nc.scalar.activation(
                        hb[:], zslot, AF.Gelu, bias=0.0, scale=Y[:, q : q + 1]
                    )
                    scr2 = scrp.tile([128, H], bf16, tag="scr2")
                    for c in range(OUT):
                        nc.vector.scalar_tensor_tensor(
                            scr2[:], hb[:], 1.0, w2sb[:, c, :],
                            ALU.mult, ALU.mult, accum_out=OB[:, q, c : c + 1],
                        )
                nc.vector.tensor_add(
                    outsb[:, u * G : (u + 1) * G, :].opt(),
                    OB[:].opt(), b2sb[:].rearrange("p (q c) -> p q c", c=OUT),
                )

            backlog = None
            for u in range(nt // G):
                if u == 0:
                    xtg = xtg_pre
                else:
                    xtg = xtp.tile([128, KC, G, 129], bf16, tag="xtg")
                    for h in range(2):
                        nc.sync.dma_start(
                            xtg[:, :, 2 * h : 2 * h + 2, :], xt_in[u, h]
                        )

                SS = statp.tile([128, G], f32, tag="SS")
                OB = statp.tile([128, G, OUT], f32, tag="OB")
                # BM cols (2q, 2q+1) = (-mu_q, rhat_q) in bf16
                BM = scrp.tile([128, 128], bf16, tag="BM")
                BMv = BM[:, 0 : 2 * G].rearrange("p (q s) -> p q s", s=2)

                pszds = []
                psgds = []
                for q in range(G):
                    if q % 2 == 0:
                        pszd = pszp.tile([128, 2, H], f32, tag="pszd")
                        pszds.append(pszd)
                        psgd = psgp.tile([128, 2, 129], f32, tag="psgd")
                        psgds.append(psgd)
                    zslot = pszds[q // 2][:, q % 2, :]
                    gslot = psgds[q // 2][:, q % 2, :]
                    # PSUM start=True clears the WHOLE 2KB bank, so a bank's
                    # two half-bank slots must share one accumulation group:
                    # only the very first matmul into the bank starts it; the
                    # odd slot's first write lands on has_written=0 elements
                    # and overwrites, later ks accumulate.
                    first = q % 2 == 0
                    for k in range(KC):
                        nc.tensor.matmul(
                            zslot, xtg[:, k, q, 0:128], w1sb[:, k, :],
                            start=(first and k == 0), stop=False,
                            skip_group_check=True,
                        )
                        nc.tensor.matmul(
                            gslot,
                            xtg[:, k, q, 0:128], xtg[:, k, q, 0:129],
                            start=(first and k == 0),
                            stop=(not first and k == KC - 1),
                            skip_group_check=True,
                        )
                if backlog is not None:
                    back_half(*backlog)

                # Extraction emitted AFTER the previous block's back half so
                # the ACT queue runs its PSUM-freeing GELUs before these
                # gram-dependent ops (-mu on ACT, diagonal on DVE).
                for q in range(G):
                    gslot = psgds[q // 2][:, q % 2, :]
                    nc.scalar.activation(
                        BMv[:, q, 0:1], gslot[:, 128:129], AF.Copy,
                        bias=0.0, scale=-1.0 / D,
                    )
                    scr = scrp.tile([128, 128], f32, tag="scr")
                    nc.vector.scalar_tensor_tensor(
                        scr[:], gslot[:, 0:128], 1.0, idfsb[:],
                        ALU.mult, ALU.mult, accum_out=SS[:, q : q + 1],
                    )

                # Batched stats: V = SS/D - mu^2 + eps; Y = rsqrt(V) via a
                # bit-trick seed + one Newton step (all-DVE — an ACT Sqrt
                # would force a per-block activation-table reload).
                A1 = statp.tile([128, G], f32, tag="A1")
                nc.vector.tensor_scalar(A1[:], SS[:], 1.0 / D, EPS, ALU.mult, ALU.add)
                B = statp.tile([128, G], f32, tag="B")
                nc.vector.tensor_tensor(B[:], BMv[:, :, 0], BMv[:, :, 0], ALU.mult)
                V = statp.tile([128, G], f32, tag="V")
                nc.vector.tensor_tensor(V[:], A1[:], B[:], ALU.subtract)
                Y = statp.tile([128, G], f32, tag="Y")
                T = statp.tile([128, G], f32, tag="T")
                nc.vector.tensor_scalar(T[:].bitcast(i32), V[:].bitcast(i32), 1, None, ALU.logical_shift_right)
                nc.vector.tensor_scalar(Y[:].bitcast(i32), T[:].bitcast(i32), -1, MAGIC, ALU.mult, ALU.add)
                for _ in range(1):
                    # one Newton step: rsqrt to ~0.2% rel (plenty for 2e-2)
                    nc.vector.tensor_tensor(T[:], V[:], Y[:], ALU.mult)
                    nc.vector.tensor_tensor(T[:], T[:], Y[:], ALU.mult)
                    nc.vector.tensor_scalar(T[:], T[:], -0.5, 1.5, ALU.mult, ALU.add)
                    nc.vector.tensor_tensor(Y[:], Y[:], T[:], ALU.mult)
                nc.vector.tensor_tensor(BMv[:, :, 1], V[:], Y[:], ALU.mult)

                # Issue the tiny stats transpose on the ACT engine's HWDGE
                # ring: on the Sync ring it head-of-line blocks the next
                # blocks' 1MB xtg prefetches behind the stats-chain dep.
                BMT = scrp.tile([128, 128], bf16, tag="BMT")
                nc.scalar.dma_start(BMT[:], BM[:], transpose=True)

                backlog = (u, pszds, BMT, Y, OB)

            back_half(*backlog)

            nc.sync.dma_start(y_v[:], outsb[:])

    nc.finalize()
    return nc


def _get_nc():
    if "nc" not in _cache:
        _cache["nc"] = _build()
    return _cache["nc"]


def _prep_weights(ln_w, ln_b, W1, b1, W2, b2):
    W1p = ln_w[:, None] * W1                      # [1024, 256]
    s1 = W1p.sum(axis=0)                          # [256]
    c1 = ln_b @ W1 + b1                           # [256]
    sc = np.zeros((2 * G, G, H), np.float32)
    for q in range(G):
        sc[2 * q, q, :] = s1
        sc[2 * q + 1, q, :] = c1
    return {
        "w1b": _bf16(W1p.reshape(KC, 128, H).transpose(1, 0, 2)),
        "screp": _bf16(sc),
        "w2rep": _bf16(np.broadcast_to(W2.T, (128, OUT, H))),
        "b2g": np.broadcast_to(np.tile(b2, G), (128, G * OUT)).astype(np.float32).copy(),
        "identf": np.eye(128, dtype=np.float32),
    }


def _make_in_maps(embedding, ln_w, ln_b, W1, b1, W2, b2):
    embedding = np.asarray(embedding, dtype=np.float32)
    weights = _prep_weights(
        np.asarray(ln_w, dtype=np.float32), np.asarray(ln_b, dtype=np.float32),
        np.asarray(W1, dtype=np.float32), np.asarray(b1, dtype=np.float32),
        np.asarray(W2, dtype=np.float32), np.asarray(b2, dtype=np.float32),
    )
    xb = _bf16(embedding)                        # bf16 cast (rounding only)
    in_maps = []
    for cid in range(N_CORES):
        xc = xb[cid * RPC : (cid + 1) * RPC]     # [RPC, 1024] bf16
        # half-block-major K-major [u, h, p, kc, j, r] + trailing ones col
        xa = np.ones((NB, 2, 128, KC, 2, 129), dtype=ml_dtypes.bfloat16)
        xa[:, :, :, :, :, 0:128] = (
            xc.T.reshape(KC, 128, NB, 2, 2, 128).transpose(2, 3, 1, 0, 4, 5)
        )
        in_maps.append({"xt": xa, **weights})
    return in_maps


def kernel(embedding, ln_w, ln_b, W1, b1, W2, b2):
    from concourse.bass_utils import run_bass_kernel_spmd

    in_maps = _make_in_maps(embedding, ln_w, ln_b, W1, b1, W2, b2)
    nc = _get_nc()
    res = run_bass_kernel_spmd(nc, in_maps, core_ids=list(range(N_CORES)))
    out = np.concatenate(
        [
            np.asarray(res.results[c]["y"]).transpose(1, 0, 2).reshape(RPC, OUT)
            for c in range(N_CORES)
        ],
        axis=0,
    )
    return out.astype(np.float32)


# revision 31
# speedup vs baseline: 1.0322x; 1.0322x over previous
"""Trainium2 Bass kernel for nn_ClassificationHead: LayerNorm -> Linear(1024,256) -> GELU -> Linear(256,2).

Data-parallel over 8 NeuronCores: each core processes 8192 rows of the
65536-row batch; the tiny weights are replicated. The host supplies each
core's shard pre-transposed in bf16 (layout-only prep: [1024, 8192] K-major
with a ones column appended per 128-row subtile for the on-device rowsum).

Per-core pipeline, per 128-row tile:
  1. TensorE: 8 accumulating matmuls x@W1' into a half-bank PSUM slot
     (PSUM tiles come in pairs [128, 2, 256] = exactly one 2KB bank
     sharing one accumulation group, doubling tile lookahead to ~12),
     plus 8 Gram matmuls reusing the same stationary x-chunk against
     [x|1] (129 cols) -> Gram + per-row sum.
  2. ACT extracts -mu (Gram ones-col, Copy w/ scale); DVE extracts
     sum(x^2) (Gram diagonal vs identity, accumulated).
  3. Per 4-tile block: var = SS/D - mu^2 + eps and Y = rsqrt(var) via
     bit-trick seed + one Newton step, all on DVE; rhat = var*Y. A
     [128,128] xbar-transpose DMA (on the ACT HWDGE ring, keeping the
     Sync ring free for input streaming) flips (-mu, rhat) into rows.
  4. TensorE adds the rank-2 correction (-mu ox s1 + rhat ox c1); the
     whole back half (rank2/GELU/dots) of each block is emitted one
     block late so the in-order PE queue never stalls on the stats
     chain (software pipelining).
  5. ACT evaluates exact GELU with per-partition scale Y -> bf16 h.
  6. DVE computes h @ W2 via two reduce dot-products; adds b2 at block end.
  7. One contiguous DMA writes the [128, 64, 2] fp32 result; the host
     untangles to [8192, 2].

Host-side weight folding (tiny, O(1MB)): W1' = ln_w[:,None]*W1,
s1 = colsum(W1'), c1 = ln_b@W1 + b1.
"""
import sys

sys.path.insert(0, "/opt/trn_rl_repo")
sys.path.insert(0, "/root/.axon_site")

import numpy as np
import ml_dtypes

N_CORES = 8
BATCH = 65536
D = 1024
H = 256
OUT = 2
RPC = BATCH // N_CORES  # rows per core
NT = RPC // 128         # 128-row tiles per core
KC = D // 128           # contraction chunks
G = 4                   # tiles per block (512 rows)
NB = NT // G            # blocks per core
EPS = 1e-5
MAGIC = 0x5F3759DF

_cache = {}


def _bf16(a):
    return np.asarray(a, dtype=ml_dtypes.bfloat16)


def _build(rpc=RPC):
    import concourse.bacc as bacc
    import concourse.mybir as mybir
    from concourse import tile

    f32 = mybir.dt.float32
    i32 = mybir.dt.int32
    bf16 = mybir.dt.bfloat16
    AF = mybir.ActivationFunctionType
    ALU = mybir.AluOpType

    nc = bacc.Bacc(None, target_bir_lowering=False, debug=False)

    nt = rpc // 128
    # Half-block-major so each load is one contiguous run per partition
    # (1KB-run slicing measured only ~12.8 GB/s per SDMA engine) and the
    # first tiles of a block can start after half the block has landed.
    xt_in = nc.dram_tensor(
        "xt", [nt // G, 2, 128, KC, 2, 129], bf16, kind="ExternalInput"
    )
    w1_in = nc.dram_tensor("w1b", [128, KC, H], bf16, kind="ExternalInput")
    sc_in = nc.dram_tensor("screp", [2 * G, G, H], bf16, kind="ExternalInput")
    w2_in = nc.dram_tensor("w2rep", [128, OUT, H], bf16, kind="ExternalInput")
    b2_in = nc.dram_tensor("b2g", [128, G * OUT], f32, kind="ExternalInput")
    idf_in = nc.dram_tensor("identf", [128, 128], f32, kind="ExternalInput")
    # Stored [p, t, c]; host untangles to [rpc, c] (contiguous DMA runs).
    y_out = nc.dram_tensor("y", [128, nt, OUT], f32, kind="ExternalOutput")
    y_v = y_out

    with tile.TileContext(nc) as tc:
        with (
            tc.tile_pool(name="wpool", bufs=1) as wp,
            tc.tile_pool(name="xtp", bufs=6) as xtp,
            tc.tile_pool(name="statp", bufs=2) as statp,
            tc.tile_pool(name="scrp", bufs=2) as scrp,
            tc.tile_pool(name="hbp", bufs=3) as hbp,
            tc.tile_pool(name="outp", bufs=1) as outp,
            tc.tile_pool(name="pszp", bufs=6, space="PSUM") as pszp,
            tc.tile_pool(name="psgp", bufs=2, space="PSUM") as psgp,
        ):
            # Prefetch the first block's activations before the weight
            # loads so the PE ramp isn't serialized behind them.
            xtg_pre = xtp.tile([128, KC, G, 129], bf16, tag="xtg")
            for h in range(2):
                nc.sync.dma_start(
                    xtg_pre[:, :, 2 * h : 2 * h + 2, :], xt_in[0, h]
                )

            w1sb = wp.tile([128, KC, H], bf16)
            nc.sync.dma_start(w1sb[:], w1_in[:])
            scsb = wp.tile([2 * G, G, H], bf16)
            nc.sync.dma_start(scsb[:], sc_in[:])
            w2sb = wp.tile([128, OUT, H], bf16)
            nc.sync.dma_start(w2sb[:], w2_in[:])
            b2sb = wp.tile([128, G * OUT], f32)
            nc.sync.dma_start(b2sb[:], b2_in[:])
            idfsb = wp.tile([128, 128], f32)
            nc.sync.dma_start(idfsb[:], idf_in[:])

            outsb = outp.tile([128, nt, OUT], f32)

            # Software pipeline: each block's back half (rank2 correction,
            # GELU, W2 dots) is EMITTED one block late so the in-order PE
            # queue never has a stats-dependent matmul ahead of the next
            # block's dense mm1/gram work (head-of-line blocking).
            def back_half(u, pszds, BMT, Y, OB):
                for q in range(G):
                    zslot = pszds[q // 2][:, q % 2, :]
                    nc.tensor.matmul(
                        zslot, BMT[0 : 2 * G, :],
                        scsb[:, q, :], start=False, stop=(q % 2 == 1),
                        skip_group_check=True,
                    )
                    hb = hbp.tile([128, H], bf16, tag="hb")
                    nc.scalar.activation(
                        hb[:], zslot, AF.Gelu, bias=0.0, scale=Y[:, q : q + 1]
                    )
                    scr2 = scrp.tile([128, H], bf16, tag="scr2")
                    for c in range(OUT):
                        nc.vector.scalar_tensor_tensor(
                            scr2[:], hb[:], 1.0, w2sb[:, c, :],
                            ALU.mult, ALU.mult, accum_out=OB[:, q, c : c + 1],
                        )
                nc.vector.tensor_add(
                    outsb[:, u * G : (u + 1) * G, :].opt(),
                    OB[:].opt(), b2sb[:].rearrange("p (q c) -> p q c", c=OUT),
                )

            backlog = None
            for u in range(nt // G):
                if u == 0:
                    xtg = xtg_pre
                else:
                    xtg = xtp.tile([128, KC, G, 129], bf16, tag="xtg")
                    for h in range(2):
                        nc.sync.dma_start(
                            xtg[:, :, 2 * h : 2 * h + 2, :], xt_in[u, h]
                        )

                SS = statp.tile([128, G], f32, tag="SS")
                OB = statp.tile([128, G, OUT], f32, tag="OB")
                # BM cols (2q, 2q+1) = (-mu_q, rhat_q) in bf16
                BM = scrp.tile([128, 128], bf16, tag="BM")
                BMv = BM[:, 0 : 2 * G].rearrange("p (q s) -> p q s", s=2)

                pszds = []
                psgds = []
                for q in range(G):
                    if q % 2 == 0:
                        pszd = pszp.tile([128, 2, H], f32, tag="pszd")
                        pszds.append(pszd)
                        psgd = psgp.tile([128, 2, 129], f32, tag="psgd")
                        psgds.append(psgd)
                    zslot = pszds[q // 2][:, q % 2, :]
                    gslot = psgds[q // 2][:, q % 2, :]
                    # PSUM start=True clears the WHOLE 2KB bank, so a bank's
                    # two half-bank slots must share one accumulation group:
                    # only the very first matmul into the bank starts it; the
                    # odd slot's first write lands on has_written=0 elements
                    # and overwrites, later ks accumulate.
                    first = q % 2 == 0
                    for k in range(KC):
                        nc.tensor.matmul(
                            zslot, xtg[:, k, q, 0:128], w1sb[:, k, :],
                            start=(first and k == 0), stop=False,
                            skip_group_check=True,
                        )
                        nc.tensor.matmul(
                            gslot,
                            xtg[:, k, q, 0:128], xtg[:, k, q, 0:129],
                            start=(first and k == 0),
                            stop=(not first and k == KC - 1),
                            skip_group_check=True,
                        )
                if backlog is not None:
                    back_half(*backlog)

                # Extraction emitted AFTER the previous block's back half so
                # the ACT queue runs its PSUM-freeing GELUs before these
                # gram-dependent ops (-mu on ACT, diagonal on DVE).
                for q in range(G):
                    gslot = psgds[q // 2][:, q % 2, :]
                    nc.scalar.activation(
                        BMv[:, q, 0:1], gslot[:, 128:129], AF.Copy,
                        bias=0.0, scale=-1.0 / D,
                    )
                    scr = scrp.tile([128, 128], f32, tag="scr")
                    nc.vector.scalar_tensor_tensor(
                        scr[:], gslot[:, 0:128], 1.0, idfsb[:],
                        ALU.mult, ALU.mult, accum_out=SS[:, q : q + 1],
                    )

                # Batched stats: V = SS/D - mu^2 + eps; Y = rsqrt(V) via a
                # bit-trick seed + one Newton step (all-DVE — an ACT Sqrt
                # would force a per-block activation-table reload).
                A1 = statp.tile([128, G], f32, tag="A1")
                nc.vector.tensor_scalar(A1[:], SS[:], 1.0 / D, EPS, ALU.mult, ALU.add)
                B = statp.tile([128, G], f32, tag="B")
                nc.vector.tensor_tensor(B[:], BMv[:, :, 0], BMv[:, :, 0], ALU.mult)
                V = statp.tile([128, G], f32, tag="V")
                nc.vector.tensor_tensor(V[:], A1[:], B[:], ALU.subtract)
                Y = statp.tile([128, G], f32, tag="Y")
                T = statp.tile([128, G], f32, tag="T")
                nc.vector.tensor_scalar(T[:].bitcast(i32), V[:].bitcast(i32), 1, None, ALU.logical_shift_right)
                nc.vector.tensor_scalar(Y[:].bitcast(i32), T[:].bitcast(i32), -1, MAGIC, ALU.mult, ALU.add)
                for _ in range(1):
                    # one Newton step: rsqrt to ~0.2% rel (plenty for 2e-2)
                    nc.vector.tensor_tensor(T[:], V[:], Y[:], ALU.mult)
                    nc.vector.tensor_tensor(T[:], T[:], Y[:], ALU.mult)
                    nc.vector.tensor_scalar(T[:], T[:], -0.5, 1.5, ALU.mult, ALU.add)
                    nc.vector.tensor_tensor(Y[:], Y[:], T[:], ALU.mult)
                nc.vector.tensor_tensor(BMv[:, :, 1], V[:], Y[:], ALU.mult)

                # Issue the tiny stats transpose on the ACT engine's HWDGE
                # ring: on the Sync ring it head-of-line blocks the next
                # blocks' 1MB xtg prefetches behind the stats-chain dep.
                BMT = scrp.tile([128, 128], bf16, tag="BMT")
                nc.scalar.dma_start(BMT[:], BM[:], transpose=True)

                backlog = (u, pszds, BMT, Y, OB)

            back_half(*backlog)

            nc.sync.dma_start(y_v[:], outsb[:])

    nc.finalize()
    return nc


def _get_nc():
    if "nc" not in _cache:
        _cache["nc"] = _build()
    return _cache["nc"]


def _prep_weights(ln_w, ln_b, W1, b1, W2, b2):
    W1p = ln_w[:, None] * W1                      # [1024, 256]
    s1 = W1p.sum(axis=0)                          # [256]
    c1 = ln_b @ W1 + b1                           # [256]
    sc = np.zeros((2 * G, G, H), np.float32)
    for q in range(G):
        sc[2 * q, q, :] = s1
        sc[2 * q + 1, q, :] = c1
    return {
        "w1b": _bf16(W1p.reshape(KC, 128, H).transpose(1, 0, 2)),
        "screp": _bf16(sc),
        "w2rep": _bf16(np.broadcast_to(W2.T, (128, OUT, H))),
        "b2g": np.broadcast_to(np.tile(b2, G), (128, G * OUT)).astype(np.float32).copy(),
        "identf": np.eye(128, dtype=np.float32),
    }


def _make_in_maps(embedding, ln_w, ln_b, W1, b1, W2, b2):
    embedding = np.asarray(embedding, dtype=np.float32)
    weights = _prep_weights(
        np.asarray(ln_w, dtype=np.float32), np.asarray(ln_b, dtype=np.float32),
        np.asarray(W1, dtype=np.float32), np.asarray(b1, dtype=np.float32),
        np.asarray(W2, dtype=np.float32), np.asarray(b2, dtype=np.float32),
    )
    xb = _bf16(embedding)                        # bf16 cast (rounding only)
    in_maps = []
    for cid in range(N_CORES):
        xc = xb[cid * RPC : (cid + 1) * RPC]     # [RPC, 1024] bf16
        # half-block-major K-major [u, h, p, kc, j, r] + trailing ones col
        xa = np.ones((NB, 2, 128, KC, 2, 129), dtype=ml_dtypes.bfloat16)
        xa[:, :, :, :, :, 0:128] = (
            xc.T.reshape(KC, 128, NB, 2, 2, 128).transpose(2, 3, 1, 0, 4, 5)
        )
        in_maps.append({"xt": xa, **weights})
    return in_maps


def kernel(embedding, ln_w, ln_b, W1, b1, W2, b2):
    from concourse.bass_utils import run_bass_kernel_spmd

    in_maps = _make_in_maps(embedding, ln_w, ln_b, W1, b1, W2, b2)
    nc = _get_nc()
    res = run_bass_kernel_spmd(nc, in_maps, core_ids=list(range(N_CORES)))
    out = np.concatenate(
        [
            np.asarray(res.results[c]["y"]).transpose(1, 0, 2).reshape(RPC, OUT)
            for c in range(N_CORES)
        ],
        axis=0,
    )
    return out.astype(np.float32)


# revision 32
# speedup vs baseline: 1.1127x; 1.0780x over previous
"""Trainium2 Bass kernel for nn_ClassificationHead: LayerNorm -> Linear(1024,256) -> GELU -> Linear(256,2).

Data-parallel over 8 NeuronCores: each core processes 8192 rows of the
65536-row batch; the tiny weights are replicated. The host supplies each
core's shard pre-transposed in bf16 (layout-only prep: [1024, 8192] K-major
with a ones column appended per 128-row subtile for the on-device rowsum).

Per-core pipeline, per 128-row tile:
  1. TensorE: 8 accumulating matmuls x@W1' into a half-bank PSUM slot
     (PSUM tiles come in pairs [128, 2, 256] = exactly one 2KB bank
     sharing one accumulation group, doubling tile lookahead to ~12),
     plus 8 Gram matmuls reusing the same stationary x-chunk against
     [x|1] (129 cols) -> Gram + per-row sum.
  2. ACT extracts -mu (Gram ones-col, Copy w/ scale); DVE extracts
     sum(x^2) (Gram diagonal vs identity, accumulated).
  3. Per 4-tile block: var = SS/D - mu^2 + eps and Y = rsqrt(var) via
     bit-trick seed + one Newton step, all on DVE; rhat = var*Y. A
     [128,128] xbar-transpose DMA (on the ACT HWDGE ring, keeping the
     Sync ring free for input streaming) flips (-mu, rhat) into rows.
  4. TensorE adds the rank-2 correction (-mu ox s1 + rhat ox c1); the
     whole back half (rank2/GELU/dots) of each block is emitted one
     block late so the in-order PE queue never stalls on the stats
     chain (software pipelining).
  5. ACT evaluates exact GELU with per-partition scale Y -> bf16 h.
  6. DVE computes h @ W2 via two reduce dot-products; adds b2 at block end.
  7. One contiguous DMA writes the [128, 64, 2] fp32 result; the host
     untangles to [8192, 2].

Host-side weight folding (tiny, O(1MB)): W1' = ln_w[:,None]*W1,
s1 = colsum(W1'), c1 = ln_b@W1 + b1.
"""
import sys

sys.path.insert(0, "/opt/trn_rl_repo")
sys.path.insert(0, "/root/.axon_site")

import numpy as np
import ml_dtypes

N_CORES = 8
BATCH = 65536
D = 1024
H = 256
OUT = 2
RPC = BATCH // N_CORES  # rows per core
NT = RPC // 128         # 128-row tiles per core
KC = D // 128           # contraction chunks
G = 4                   # tiles per block (512 rows)
NB = NT // G            # blocks per core
EPS = 1e-5
MAGIC = 0x5F3759DF

_cache = {}


def _bf16(a):
    return np.asarray(a, dtype=ml_dtypes.bfloat16)


def _build(rpc=RPC):
    import concourse.bacc as bacc
    import concourse.mybir as mybir
    from concourse import tile

    f32 = mybir.dt.float32
    i32 = mybir.dt.int32
    bf16 = mybir.dt.bfloat16
    AF = mybir.ActivationFunctionType
    ALU = mybir.AluOpType

    nc = bacc.Bacc(None, target_bir_lowering=False, debug=False)

    nt = rpc // 128
    # Half-block-major so each load is one contiguous run per partition
    # (1KB-run slicing measured only ~12.8 GB/s per SDMA engine) and the
    # first tiles of a block can start after half the block has landed.
    xt_in = nc.dram_tensor(
        "xt", [nt // G, 2, 128, KC, 2, 129], bf16, kind="ExternalInput"
    )
    w1_in = nc.dram_tensor("w1b", [128, KC, H], bf16, kind="ExternalInput")
    sc_in = nc.dram_tensor("screp", [2 * G, G, H], bf16, kind="ExternalInput")
    w2_in = nc.dram_tensor("w2rep", [128, OUT, H], bf16, kind="ExternalInput")
    b2_in = nc.dram_tensor("b2g", [128, G * OUT], f32, kind="ExternalInput")
    idf_in = nc.dram_tensor("identf", [128, 128], f32, kind="ExternalInput")
    # Stored [p, t, c]; host untangles to [rpc, c] (contiguous DMA runs).
    y_out = nc.dram_tensor("y", [128, nt, OUT], f32, kind="ExternalOutput")
    y_v = y_out

    with tile.TileContext(nc) as tc:
        with (
            tc.tile_pool(name="wpool", bufs=1) as wp,
            tc.tile_pool(name="xtp", bufs=6) as xtp,
            tc.tile_pool(name="statp", bufs=2) as statp,
            tc.tile_pool(name="scrp", bufs=2) as scrp,
            tc.tile_pool(name="hbp", bufs=3) as hbp,
            tc.tile_pool(name="outp", bufs=1) as outp,
            tc.tile_pool(name="pszp", bufs=6, space="PSUM") as pszp,
            tc.tile_pool(name="psgp", bufs=2, space="PSUM") as psgp,
        ):
            # Prefetch the first block's activations before the weight
            # loads so the PE ramp isn't serialized behind them.
            xtg_pre = xtp.tile([128, KC, G, 129], bf16, tag="xtg")
            for h in range(2):
                nc.sync.dma_start(
                    xtg_pre[:, :, 2 * h : 2 * h + 2, :], xt_in[0, h]
                )

            w1sb = wp.tile([128, KC, H], bf16)
            nc.sync.dma_start(w1sb[:], w1_in[:])
            scsb = wp.tile([2 * G, G, H], bf16)
            nc.sync.dma_start(scsb[:], sc_in[:])
            w2sb = wp.tile([128, OUT, H], bf16)
            nc.sync.dma_start(w2sb[:], w2_in[:])
            b2sb = wp.tile([128, G * OUT], f32)
            nc.sync.dma_start(b2sb[:], b2_in[:])
            idfsb = wp.tile([128, 128], f32)
            nc.sync.dma_start(idfsb[:], idf_in[:])

            outsb = outp.tile([128, nt, OUT], f32)

            # Software pipeline: each block's back half (rank2 correction,
            # GELU, W2 dots) is EMITTED one block late so the in-order PE
            # queue never has a stats-dependent matmul ahead of the next
            # block's dense mm1/gram work (head-of-line blocking).
            def back_half(u, pszds, BMT, Y, OB):
                for q in range(G):
                    zslot = pszds[q // 2][:, q % 2, :]
                    nc.tensor.matmul(
                        zslot, BMT[0 : 2 * G, :],
                        scsb[:, q, :], start=False, stop=(q % 2 == 1),
                        skip_group_check=True,
                    )
                    hb = hbp.tile([128, H], bf16, tag="hb")
                    nc.scalar.activation(
                        hb[:], zslot, AF.Gelu, bias=0.0, scale=Y[:, q : q + 1]
                    )
                    scr2 = scrp.tile([128, H], bf16, tag="scr2")
                    for c in range(OUT):
                        nc.vector.scalar_tensor_tensor(
                            scr2[:], hb[:], 1.0, w2sb[:, c, :],
                            ALU.mult, ALU.mult, accum_out=OB[:, q, c : c + 1],
                        )
                nc.vector.tensor_add(
                    outsb[:, u * G : (u + 1) * G, :].opt(),
                    OB[:].opt(), b2sb[:].rearrange("p (q c) -> p q c", c=OUT),
                )

            backlog = None
            for u in range(nt // G):
                if u == 0:
                    xtg = xtg_pre
                else:
                    xtg = xtp.tile([128, KC, G, 129], bf16, tag="xtg")
                    for h in range(2):
                        # Alternate HWDGE rings so one ring's buffer-reuse
                        # wait can't head-of-line block the other's issues.
                        eng = nc.sync if h == 0 else nc.scalar
                        eng.dma_start(
                            xtg[:, :, 2 * h : 2 * h + 2, :], xt_in[u, h]
                        )

                SS = statp.tile([128, G], f32, tag="SS")
                OB = statp.tile([128, G, OUT], f32, tag="OB")
                # BM cols (2q, 2q+1) = (-mu_q, rhat_q) in bf16
                BM = scrp.tile([128, 128], bf16, tag="BM")
                BMv = BM[:, 0 : 2 * G].rearrange("p (q s) -> p q s", s=2)

                pszds = []
                psgds = []
                for q in range(G):
                    if q % 2 == 0:
                        pszd = pszp.tile([128, 2, H], f32, tag="pszd")
                        pszds.append(pszd)
                        psgd = psgp.tile([128, 2, 129], f32, tag="psgd")
                        psgds.append(psgd)
                    zslot = pszds[q // 2][:, q % 2, :]
                    gslot = psgds[q // 2][:, q % 2, :]
                    # PSUM start=True clears the WHOLE 2KB bank, so a bank's
                    # two half-bank slots must share one accumulation group:
                    # only the very first matmul into the bank starts it; the
                    # odd slot's first write lands on has_written=0 elements
                    # and overwrites, later ks accumulate.
                    first = q % 2 == 0
                    for k in range(KC):
                        nc.tensor.matmul(
                            zslot, xtg[:, k, q, 0:128], w1sb[:, k, :],
                            start=(first and k == 0), stop=False,
                            skip_group_check=True,
                        )
                        nc.tensor.matmul(
                            gslot,
                            xtg[:, k, q, 0:128], xtg[:, k, q, 0:129],
                            start=(first and k == 0),
                            stop=(not first and k == KC - 1),
                            skip_group_check=True,
                        )
                if backlog is not None:
                    back_half(*backlog)

                # Extraction emitted AFTER the previous block's back half so
                # the ACT queue runs its PSUM-freeing GELUs before these
                # gram-dependent ops (-mu on ACT, diagonal on DVE).
                for q in range(G):
                    gslot = psgds[q // 2][:, q % 2, :]
                    nc.scalar.activation(
                        BMv[:, q, 0:1], gslot[:, 128:129], AF.Copy,
                        bias=0.0, scale=-1.0 / D,
                    )
                    scr = scrp.tile([128, 128], f32, tag="scr")
                    nc.vector.scalar_tensor_tensor(
                        scr[:], gslot[:, 0:128], 1.0, idfsb[:],
                        ALU.mult, ALU.mult, accum_out=SS[:, q : q + 1],
                    )

                # Batched stats: V = SS/D - mu^2 + eps; Y = rsqrt(V) via a
                # bit-trick seed + one Newton step (all-DVE — an ACT Sqrt
                # would force a per-block activation-table reload).
                A1 = statp.tile([128, G], f32, tag="A1")
                nc.vector.tensor_scalar(A1[:], SS[:], 1.0 / D, EPS, ALU.mult, ALU.add)
                B = statp.tile([128, G], f32, tag="B")
                nc.vector.tensor_tensor(B[:], BMv[:, :, 0], BMv[:, :, 0], ALU.mult)
                V = statp.tile([128, G], f32, tag="V")
                nc.vector.tensor_tensor(V[:], A1[:], B[:], ALU.subtract)
                Y = statp.tile([128, G], f32, tag="Y")
                T = statp.tile([128, G], f32, tag="T")
                nc.vector.tensor_scalar(T[:].bitcast(i32), V[:].bitcast(i32), 1, None, ALU.logical_shift_right)
                nc.vector.tensor_scalar(Y[:].bitcast(i32), T[:].bitcast(i32), -1, MAGIC, ALU.mult, ALU.add)
                for _ in range(1):
                    # one Newton step: rsqrt to ~0.2% rel (plenty for 2e-2)
                    nc.vector.tensor_tensor(T[:], V[:], Y[:], ALU.mult)
                    nc.vector.tensor_tensor(T[:], T[:], Y[:], ALU.mult)
                    nc.vector.tensor_scalar(T[:], T[:], -0.5, 1.5, ALU.mult, ALU.add)
                    nc.vector.tensor_tensor(Y[:], Y[:], T[:], ALU.mult)
                nc.vector.tensor_tensor(BMv[:, :, 1], V[:], Y[:], ALU.mult)

                # Issue the tiny stats transpose on the ACT engine's HWDGE
                # ring: on the Sync ring it head-of-line blocks the next
                # blocks' 1MB xtg prefetches behind the stats-chain dep.
                BMT = scrp.tile([128, 128], bf16, tag="BMT")
                nc.scalar.dma_start(BMT[:], BM[:], transpose=True)

                backlog = (u, pszds, BMT, Y, OB)

            back_half(*backlog)

            nc.sync.dma_start(y_v[:], outsb[:])

    nc.finalize()
    return nc


def _get_nc():
    if "nc" not in _cache:
        _cache["nc"] = _build()
    return _cache["nc"]


def _prep_weights(ln_w, ln_b, W1, b1, W2, b2):
    W1p = ln_w[:, None] * W1                      # [1024, 256]
    s1 = W1p.sum(axis=0)                          # [256]
    c1 = ln_b @ W1 + b1                           # [256]
    sc = np.zeros((2 * G, G, H), np.float32)
    for q in range(G):
        sc[2 * q, q, :] = s1
        sc[2 * q + 1, q, :] = c1
    return {
        "w1b": _bf16(W1p.reshape(KC, 128, H).transpose(1, 0, 2)),
        "screp": _bf16(sc),
        "w2rep": _bf16(np.broadcast_to(W2.T, (128, OUT, H))),
        "b2g": np.broadcast_to(np.tile(b2, G), (128, G * OUT)).astype(np.float32).copy(),
        "identf": np.eye(128, dtype=np.float32),
    }


def _make_in_maps(embedding, ln_w, ln_b, W1, b1, W2, b2):
    embedding = np.asarray(embedding, dtype=np.float32)
    weights = _prep_weights(
        np.asarray(ln_w, dtype=np.float32), np.asarray(ln_b, dtype=np.float32),
        np.asarray(W1, dtype=np.float32), np.asarray(b1, dtype=np.float32),
        np.asarray(W2, dtype=np.float32), np.asarray(b2, dtype=np.float32),
    )
    xb = _bf16(embedding)                        # bf16 cast (rounding only)
    in_maps = []
    for cid in range(N_CORES):
        xc = xb[cid * RPC : (cid + 1) * RPC]     # [RPC, 1024] bf16
        # half-block-major K-major [u, h, p, kc, j, r] + trailing ones col
        xa = np.ones((NB, 2, 128, KC, 2, 129), dtype=ml_dtypes.bfloat16)
        xa[:, :, :, :, :, 0:128] = (
            xc.T.reshape(KC, 128, NB, 2, 2, 128).transpose(2, 3, 1, 0, 4, 5)
        )
        in_maps.append({"xt": xa, **weights})
    return in_maps


def kernel(embedding, ln_w, ln_b, W1, b1, W2, b2):
    from concourse.bass_utils import run_bass_kernel_spmd

    in_maps = _make_in_maps(embedding, ln_w, ln_b, W1, b1, W2, b2)
    nc = _get_nc()
    res = run_bass_kernel_spmd(nc, in_maps, core_ids=list(range(N_CORES)))
    out = np.concatenate(
        [
            np.asarray(res.results[c]["y"]).transpose(1, 0, 2).reshape(RPC, OUT)
            for c in range(N_CORES)
        ],
        axis=0,
    )
    return out.astype(np.float32)
